# revision 1
# baseline (speedup 1.0000x reference)
"""Trainium2 Bass kernel for octonion causal self-attention.

Sharding: 8 cores = 4 batches x 2 head-groups. Core c handles batch b=c//2 and
head-group g=c%2 (octonion output components 4g..4g+3 = heads 8g..8g+7).
Each core computes q/k/v projections for its components from the full x[b],
RoPE, causal attention for its 8 heads, the octonion head-mixer for its group,
and a partial wo projection (its 4 input components, all 2048 output channels).
The host sums the two partials per batch and transposes. No collectives.

All activations flow through the TensorEngine transposed ([feature, token])
so every matmul contraction lands on the partition dim with no on-device
transposes of x. Matmuls run as float32r (fp22 multiply, fp32 accumulate,
full PE rate at free-dim >= 256).

Weight prep on host: BitNet ternary quantization replicated exactly (jax on
CPU, matching the reference's jnp ops bit-for-bit), octonion sign/index tables
folded into per-(i,j) expanded blocks, attention scale folded into wq, and
mixer_beta folded into the mixer weights. q/k output dims are permuted
evens-first per head so RoPE's pair rotation becomes two contiguous
half-partition blocks.
"""

import math
import os
from contextlib import ExitStack

import numpy as np

B, T, C, H, D = 4, 1024, 2048, 16, 128
C8 = C // 8  # 256
NCORES = 8
P = 128
NEG = -1.0e30


# ---------------- octonion tables (matches reference) ----------------
def _cd_conj(a):
    n = a.shape[0]
    if n == 1:
        return a
    h = n // 2
    return np.concatenate([_cd_conj(a[:h]), -a[h:]])


def _cd_mul(a, b):
    n = a.shape[0]
    if n == 1:
        return a * b
    h = n // 2
    a1, a2 = a[:h], a[h:]
    c1, c2 = b[:h], b[h:]
    return np.concatenate(
        [
            _cd_mul(a1, c1) - _cd_mul(_cd_conj(c2), a2),
            _cd_mul(c2, a1) + _cd_mul(a2, _cd_conj(c1)),
        ]
    )


def _octonion_tables():
    signs = np.zeros((8, 8), dtype=np.float32)
    widx = np.zeros((8, 8), dtype=np.int32)
    for i in range(8):
        for j in range(8):
            ei = np.zeros(8)
            ei[i] = 1.0
            ej = np.zeros(8)
            ej[j] = 1.0
            p = _cd_mul(ei, ej)
            k = int(np.argmax(np.abs(p)))
            signs[i, j] = np.sign(p[k])
            widx[i, j] = k
    return signs, widx


SIGNS, WIDX = _octonion_tables()


def _ternary_quantize(W: np.ndarray) -> np.ndarray:
    """Replicates reference ternary_ste forward pass bit-exactly (jnp on CPU)."""
    import jax
    import jax.numpy as jnp

    with jax.default_device(jax.devices("cpu")[0]):
        Wj = jnp.asarray(W)
        s = jnp.mean(jnp.abs(Wj), axis=(-2, -1), keepdims=True) + 1e-8
        Wq = jnp.clip(jnp.round(Wj / s), -1.0, 1.0) * s
        return np.asarray(Wq)


def _signed_full(Wq: np.ndarray, i: int) -> np.ndarray:
    """[2048, 256] block column for octonion output component i:
    rows j*256:(j+1)*256 = SIGNS[i,j] * Wq[i^j]."""
    out = np.empty((C, C8), dtype=np.float32)
    for j in range(8):
        out[j * C8 : (j + 1) * C8, :] = SIGNS[i, j] * Wq[i ^ j]
    return out


_EVENS_FIRST = np.concatenate([np.arange(0, D, 2), np.arange(1, D, 2)])


def _prep_core_inputs(
    inputs: dict, b: int, g: int, wq_q, wk_q, wv_q, wo_q, rope_dma: bool = False
):
    x = inputs["x"]
    fc, fs = inputs["freqs_cos"], inputs["freqs_sin"]
    mixer_W, mixer_beta = inputs["mixer_W"], inputs["mixer_beta"]

    m = {}
    # x transposed: [ct, p, t]
    m["xT"] = np.ascontiguousarray(x[b].T).reshape(16, P, T)

    # q/k weights: [qk, li, dh, c_p, ct, d2], evens-first permuted output dims
    wqk = np.empty((2, 4, 2, P, 16, P), dtype=np.float32)
    qscale = np.float32(1.0 / math.sqrt(D))
    for qk, Wq in enumerate((wq_q, wk_q)):
        for li in range(4):
            i = 4 * g + li
            Bf = _signed_full(Wq, i)  # [2048, 256]
            if qk == 0:
                Bf = Bf * qscale
            for dh in range(2):
                Bh = Bf[:, dh * D : (dh + 1) * D]  # [2048, 128]
                if rope_dma:
                    Bh = Bh[:, _EVENS_FIRST]
                # [c, d2] -> [c_p, ct, d2]
                wqk[qk, li, dh] = Bh.reshape(16, P, P).transpose(1, 0, 2)
    m["wqk"] = wqk

    # v weights: [lp, ct, c_p, dcol] with dcol = 2 comps x 256, natural order
    wv = np.empty((2, 16, P, 512), dtype=np.float32)
    for lp in range(2):
        B2 = np.concatenate(
            [_signed_full(wv_q, 4 * g + 2 * lp + u) for u in range(2)], axis=1
        )  # [2048, 512]
        wv[lp] = B2.reshape(16, P, 512)
    m["wv"] = wv

    # wo: [ft, d_p, kt, f] ; kt = local input channel tile (4 comps x 2 halves)
    wo = np.empty((16, P, 8, P), dtype=np.float32)
    for ft in range(16):
        i_o, fh = ft // 2, ft % 2
        for kt in range(8):
            j = 4 * g + kt // 2
            dloc = kt % 2
            blk = SIGNS[i_o, j] * wo_q[i_o ^ j]  # [256, 256]
            wo[ft, :, kt, :] = blk[dloc * P : (dloc + 1) * P, fh * P : (fh + 1) * P]
    m["wo"] = wo

    # mixer: [i, d_p, j, e], beta folded on output dim
    wm = np.empty((8, P, 8, P), dtype=np.float32)
    for i in range(8):
        for j in range(8):
            wm[i, :, j, :] = (SIGNS[i, j] * mixer_W[i ^ j]) * mixer_beta[None, :]
    m["wm"] = wm

    if rope_dma:
        # evens-first layout: rows 0..63 even dims (freq p), 64..127 odd dims.
        # rope(q')[p] = q'[p]*A[p] + q'[p xor 64]*B[p]; A = cos, B = -sin | +sin
        cosP = np.ascontiguousarray(fc.T)  # [64, 1024]
        sinP = np.ascontiguousarray(fs.T)
        m["cosd"] = np.concatenate([cosP, cosP], axis=0)
        m["sind"] = np.concatenate([-sinP, sinP], axis=0)
        m["pswap"] = np.zeros((P, P), dtype=np.float32)  # unused
    else:
        # interleaved layout: row p uses freq p//2; partner = p xor 1 via PE
        cosA = np.repeat(np.ascontiguousarray(fc.T), 2, axis=0)
        sinB = np.repeat(np.ascontiguousarray(fs.T), 2, axis=0)
        sinB[0::2, :] *= -1.0
        m["cosd"] = cosA
        m["sind"] = sinB
        pswap = np.zeros((P, P), dtype=np.float32)
        for p in range(P):
            pswap[p ^ 1, p] = 1.0
        m["pswap"] = pswap
    m["ident"] = np.eye(P, dtype=np.float32)

    # causal masks for the diagonal 512-chunk: [4, 128, 512]
    pidx = np.arange(P)[:, None]
    fidx = np.arange(512)[None, :]
    mask = np.stack(
        [
            np.where(fidx <= mm * P + pidx, 0.0, NEG).astype(np.float32)
            for mm in range(4)
        ]
    )
    m["mask"] = mask
    return m


# ---------------- device program ----------------
_NC_CACHE = {}


def _build_nc(
    repeat: int = 1,
    ct_outer: bool = True,
    split_dma: bool = True,
    rope_dma: bool = False,
    attn_pipe: bool = False,
    tune2: bool = True,
    tune3: bool = True,
):
    key = (repeat, ct_outer, split_dma, rope_dma, attn_pipe, tune2, tune3)
    if key in _NC_CACHE:
        return _NC_CACHE[key]

    import concourse.mybir as mybir
    import concourse.tile as tile
    from concourse import bacc

    dt = mybir.dt
    ALU = mybir.AluOpType
    AF = mybir.ActivationFunctionType
    f32, f32r = dt.float32, dt.float32r

    nc = bacc.Bacc("TRN2", target_bir_lowering=False)

    xT = nc.declare_dram_parameter("xT", [16, P, T], f32, isOutput=False)
    wqk = nc.declare_dram_parameter("wqk", [2, 4, 2, P, 16, P], f32, isOutput=False)
    wv = nc.declare_dram_parameter("wv", [2, 16, P, 512], f32, isOutput=False)
    wo = nc.declare_dram_parameter("wo", [16, P, 8, P], f32, isOutput=False)
    wm = nc.declare_dram_parameter("wm", [8, P, 8, P], f32, isOutput=False)
    cosd = nc.declare_dram_parameter("cosd", [P, T], f32, isOutput=False)
    sind = nc.declare_dram_parameter("sind", [P, T], f32, isOutput=False)
    maskp = nc.declare_dram_parameter("mask", [4, P, 512], f32, isOutput=False)
    identp = nc.declare_dram_parameter("ident", [P, P], f32, isOutput=False)
    pswapp = nc.declare_dram_parameter("pswap", [P, P], f32, isOutput=False)
    outT = nc.declare_dram_parameter("outT", [C, T], f32, isOutput=True)

    def _eng(nc, idx):
        return nc.sync if (idx % 2 == 0 or not split_dma) else nc.gpsimd

    with tile.TileContext(nc) as tc, ExitStack() as ctx:
        cst = ctx.enter_context(tc.tile_pool(name="cst", bufs=1))
        stat_pool = ctx.enter_context(
            tc.tile_pool(name="statp", bufs=8)
        )
        stage_pool = ctx.enter_context(tc.tile_pool(name="stagep", bufs=2))
        dram = ctx.enter_context(tc.tile_pool(name="drampool", bufs=1, space="DRAM"))
        ps_proj = ctx.enter_context(
            tc.tile_pool(name="psproj", bufs=2 if tune2 else 3, space="PSUM")
        )
        ps_s = ctx.enter_context(tc.tile_pool(name="pss", bufs=2, space="PSUM"))
        ps_t = ctx.enter_context(tc.tile_pool(name="pst", bufs=2, space="PSUM"))
        ps_y = ctx.enter_context(
            tc.tile_pool(name="psy", bufs=2 if tune2 else 1, space="PSUM")
        )

        for _rep in range(repeat):
            # phase-scoped pools, opened/closed explicitly (LIFO) to bound peak SBUF
            qks_cm = tc.tile_pool(name="qks", bufs=1)
            qks = qks_cm.__enter__()
            qT_h = [qks.tile([P, T], f32, tag=f"qT{i}", name=f"qTh{i}") for i in range(8)]
            kT_h = [qks.tile([P, T], f32, tag=f"kT{i}", name=f"kTh{i}") for i in range(8)]
            xp_cm = tc.tile_pool(name="xp", bufs=1)
            xp = xp_cm.__enter__()
            wv_cm = tc.tile_pool(name="wvp", bufs=16)
            wv_pool = wv_cm.__enter__()

            # constants (small, on the gpsimd SWDGE queue to keep HWDGE free)
            ceng = nc.gpsimd if split_dma else nc.sync
            ident = cst.tile([P, P], f32, tag="ident")
            ceng.dma_start(ident[:].bitcast(f32r), identp[:].bitcast(f32r))
            cos_sb = cst.tile([P, T], f32, tag="cos")
            sin_sb = cst.tile([P, T], f32, tag="sin")
            ceng.dma_start(cos_sb[:], cosd[:])
            ceng.dma_start(sin_sb[:], sind[:])
            mask_sb = cst.tile([P, 4, 512], f32, tag="mask")
            ceng.dma_start(mask_sb[:], maskp[:].rearrange("m p f -> p m f"))
            zeros_sb = cst.tile([P, 512], f32, tag="zeros")
            nc.gpsimd.memset(zeros_sb[:], 0.0)
            if not rope_dma:
                pswap_sb = cst.tile([P, P], f32, tag="pswap")
                ceng.dma_start(pswap_sb[:].bitcast(f32r), pswapp[:].bitcast(f32r))

            # x resident, transposed; interleave x and wv loads across both
            # DMA queue families so the first V psum group completes ASAP
            x_t = [xp.tile([P, T], f32, tag=f"xT{i}", name=f"xt{i}") for i in range(16)]
            wv_tiles = {}
            if split_dma:
                start_engs = [nc.sync, nc.gpsimd, nc.scalar]
            else:
                start_engs = [nc.sync]
            for ct in range(16):
                eng = start_engs[ct % len(start_engs)]
                wt = wv_pool.tile([P, 512], f32, tag="wv", name=f"wv0_{ct}")
                eng.dma_start(wt[:].bitcast(f32r), wv[0, ct].bitcast(f32r))
                wv_tiles[0, ct] = wt
                eng2 = start_engs[(ct + 1) % len(start_engs)]
                eng2.dma_start(x_t[ct][:].bitcast(f32r), xT[ct].bitcast(f32r))
            for ct in range(16):
                eng = _eng(nc, ct)
                wt = wv_pool.tile([P, 512], f32, tag="wv", name=f"wv1_{ct}")
                eng.dma_start(wt[:].bitcast(f32r), wv[1, ct].bitcast(f32r))
                wv_tiles[1, ct] = wt

            v_dram = dram.tile([T, 8 * P], f32, tag="vdram")  # [t, d]

            # ---- V projection (spilled to DRAM scratch) ----
            for lp in range(2):
                for tt in range(8):
                    vps = ps_proj.tile([P, 512], f32, tag="proj")
                    for ct in range(16):
                        nc.tensor.matmul(
                            vps[:],
                            x_t[ct][:, tt * P : (tt + 1) * P].bitcast(f32r),
                            wv_tiles[lp, ct][:].bitcast(f32r),
                            start=(ct == 0),
                            stop=(ct == 15),
                        )
                    vsb = stage_pool.tile([P, 512], f32, tag="vstage")
                    nc.any.tensor_copy(out=vsb[:], in_=vps[:])
                    eng = _eng(nc, tt)
                    eng.dma_start(
                        v_dram[tt * P : (tt + 1) * P, lp * 512 : (lp + 1) * 512],
                        vsb[:],
                    )

            wv_cm.__exit__(None, None, None)
            wqk_cm = tc.tile_pool(name="wqkp", bufs=4 if tune2 else 3)
            wqk_pool = wqk_cm.__enter__()
            rope_cm = tc.tile_pool(name="ropep", bufs=4)
            rope_pool = rope_cm.__enter__()

            # ---- Q/K projections with fused RoPE ----
            # ct-outer with both t-chunk accumulators live so consecutive
            # matmuls share the stationary weight tile
            for qk, dest_h in ((0, qT_h), (1, kT_h)):
                for li in range(4):
                    for dh in range(2):
                        wt = wqk_pool.tile([P, 16, P], f32, tag="wqk")
                        eng = _eng(nc, li * 2 + dh)
                        eng.dma_start(
                            wt[:].bitcast(f32r), wqk[qk, li, dh].bitcast(f32r)
                        )
                        hh = li * 2 + dh
                        pps = [
                            ps_proj.tile([P, 512], f32, tag="proj", name=f"pp{t}")
                            for t in range(2)
                        ]
                        if ct_outer:
                            for ct in range(16):
                                for tci in range(2):
                                    nc.tensor.matmul(
                                        pps[tci][:],
                                        wt[:, ct, :].bitcast(f32r),
                                        x_t[ct][:, tci * 512 : (tci + 1) * 512].bitcast(f32r),
                                        start=(ct == 0),
                                        stop=(ct == 15),
                                    )
                        else:
                            for tci in range(2):
                                for ct in range(16):
                                    nc.tensor.matmul(
                                        pps[tci][:],
                                        wt[:, ct, :].bitcast(f32r),
                                        x_t[ct][:, tci * 512 : (tci + 1) * 512].bitcast(f32r),
                                        start=(ct == 0),
                                        stop=(ct == 15),
                                    )
                        for tci in range(2):
                            tsl = slice(tci * 512, (tci + 1) * 512)
                            qsb = rope_pool.tile([P, 512], f32, tag="qsb")
                            if rope_dma:
                                nc.any.tensor_copy(out=qsb[:], in_=pps[tci][:])
                                qsw = rope_pool.tile([P, 512], f32, tag="qsw")
                                eng = _eng(nc, li * 2 + dh + tci)
                                eng.dma_start(qsw[0:64, :], qsb[64:128, :])
                                eng.dma_start(qsw[64:128, :], qsb[0:64, :])
                            else:
                                nc.any.tensor_copy(
                                    out=qsb[:].bitcast(f32r), in_=pps[tci][:]
                                )
                                qsw = ps_t.tile([P, 512], f32, tag="tp", name="swps")
                                nc.tensor.matmul(
                                    qsw[:],
                                    pswap_sb[:].bitcast(f32r),
                                    qsb[:].bitcast(f32r),
                                    start=True,
                                    stop=True,
                                )
                            t1 = rope_pool.tile([P, 512], f32, tag="t1")
                            t2 = rope_pool.tile([P, 512], f32, tag="t2")
                            nc.vector.tensor_tensor(
                                t1[:], qsb[:], cos_sb[:, tsl], ALU.mult
                            )
                            nc.vector.tensor_tensor(
                                t2[:], qsw[:], sin_sb[:, tsl], ALU.mult
                            )
                            nc.vector.tensor_tensor(
                                dest_h[hh][:, tsl].bitcast(f32r), t1[:], t2[:], ALU.add
                            )

            rope_cm.__exit__(None, None, None)
            wqk_cm.__exit__(None, None, None)
            xp_cm.__exit__(None, None, None)
            yp_cm = tc.tile_pool(name="yp", bufs=1)
            yp = yp_cm.__enter__()
            p_cm = tc.tile_pool(name="ppool", bufs=5)
            p_pool = p_cm.__enter__()
            pt_cm = tc.tile_pool(name="ptpool", bufs=10)
            pt_pool = pt_cm.__enter__()
            vh_cm = tc.tile_pool(name="vhp", bufs=3 if tune3 else 2)
            vh_pool = vh_cm.__enter__()
            w2_cm = tc.tile_pool(name="w2p", bufs=2)
            w2_pool = w2_cm.__enter__()
            zp_cm = tc.tile_pool(name="zp", bufs=1)
            zp = zp_cm.__enter__()

            # ---- attention, pipelined with mixer+wo per 512-token chunk ----
            for qc in range(2):
                nkt = 4 * (qc + 1)
                y_sb = yp.tile([P, 8, 512], f32, tag="y", name=f"ysb{qc}")
                z_sb = zp.tile([P, 8, 512], f32, tag="z", name=f"zsb{qc}")
                def emit_transposes(h, qts, Ps_list, PTs):
                    for qt in qts:
                        Ps = Ps_list[qt]
                        for kt in range(qt + 1):
                            tps = ps_t.tile([P, P], f32, tag="tp")
                            nc.tensor.transpose(
                                tps[:].bitcast(f32r),
                                Ps[:, kt * P : (kt + 1) * P].bitcast(f32r),
                                ident[:].bitcast(f32r),
                            )
                            nc.any.tensor_copy(
                                out=PTs[kt][
                                    :, (qt % 4) * P : (qt % 4 + 1) * P
                                ].bitcast(f32r),
                                in_=tps[:],
                            )

                def emit_pv(h, vh, PTs):
                    yps = ps_y.tile([P, 512], f32, tag="y")
                    for kt in range(nkt):
                        nc.tensor.matmul(
                            yps[:],
                            vh[:, kt, :].bitcast(f32r),
                            PTs[kt][:].bitcast(f32r),
                            start=(kt == 0),
                            stop=(kt == nkt - 1),
                        )
                    nc.any.tensor_copy(
                        out=y_sb[:, h, :].bitcast(f32r), in_=yps[:]
                    )

                pending = None
                for h in range(8):
                    vh = vh_pool.tile([P, 8, P], f32, tag="vh")
                    eng = _eng(nc, h)
                    eng.dma_start(
                        vh[:, :nkt, :].bitcast(f32r),
                        v_dram[: nkt * P]
                        .rearrange("(kt p) d -> p kt d", p=P)[
                            :, :, h * P : (h + 1) * P
                        ]
                        .bitcast(f32r),
                    )
                    PTs = [
                        pt_pool.tile([P, 512], f32, tag="PT", name=f"PT{h}_{qc}_{ii}")
                        for ii in range(nkt)
                    ]
                    for kt in range(4 * qc, nkt):
                        zw = (kt - 4 * qc) * P
                        if zw > 0:
                            nc.vector.tensor_copy(
                                out=PTs[kt][:, :zw].bitcast(f32r),
                                in_=zeros_sb[:, :zw],
                            )
                    Ps_list = {}
                    for qt in range(4 * qc, 4 * qc + 4):
                        nch = qt // 4 + 1
                        Ps = p_pool.tile([P, T], f32, tag="P", name=f"Ps{h}_{qt}")
                        Ps_list[qt] = Ps
                        lparts = []
                        for chi in range(nch):
                            csl = slice(chi * 512, (chi + 1) * 512)
                            sps = ps_s.tile([P, 512], f32, tag="S", name=f"sps{h}_{qt}_{chi}")
                            nc.tensor.matmul(
                                sps[:],
                                qT_h[h][:, qt * P : (qt + 1) * P].bitcast(f32r),
                                kT_h[h][:, csl].bitcast(f32r),
                                start=True,
                                stop=True,
                            )
                            if chi == nch - 1:
                                nc.vector.tensor_tensor(
                                    sps[:], sps[:], mask_sb[:, qt % 4, :], ALU.add
                                )
                            lt = stat_pool.tile([P, 1], f32, tag="l")
                            nc.scalar.activation(
                                Ps[:, csl].bitcast(f32r),
                                sps[:],
                                AF.Exp,
                                accum_out=lt[:],
                            )
                            lparts.append(lt)
                        if nch == 2:
                            ltot = stat_pool.tile([P, 1], f32, tag="l2")
                            nc.vector.tensor_tensor(
                                ltot[:], lparts[0][:], lparts[1][:], ALU.add
                            )
                        else:
                            ltot = lparts[0]
                        rec = stat_pool.tile([P, 1], f32, tag="r")
                        nc.vector.reciprocal(rec[:], ltot[:])
                        w = nch * 512
                        nc.vector.tensor_scalar(
                            Ps[:, :w].bitcast(f32r),
                            Ps[:, :w],
                            rec[:],
                            None,
                            op0=ALU.mult,
                        )
                    if attn_pipe:
                        if pending is not None:
                            ph, pvh, pPTs, pPs = pending
                            emit_transposes(
                                ph, list(range(4 * qc + 2, 4 * qc + 4)), pPs, pPTs
                            )
                            emit_pv(ph, pvh, pPTs)
                        emit_transposes(
                            h, list(range(4 * qc, 4 * qc + 2)), Ps_list, PTs
                        )
                        pending = (h, vh, PTs, Ps_list)
                    else:
                        emit_transposes(
                            h, list(range(4 * qc, 4 * qc + 4)), Ps_list, PTs
                        )
                        emit_pv(h, vh, PTs)

                if attn_pipe:
                    ph, pvh, pPTs, pPs = pending
                    emit_transposes(
                        ph, list(range(4 * qc + 2, 4 * qc + 4)), pPs, pPTs
                    )
                    emit_pv(ph, pvh, pPTs)

                tsl = slice(qc * 512, (qc + 1) * 512)

                # ---- head mixer for this t-chunk ----
                for i in range(8):
                    wmt = w2_pool.tile([P, 8, P], f32, tag="wm", name=f"wm{qc}_{i}")
                    eng = _eng(nc, i)
                    eng.dma_start(wmt[:].bitcast(f32r), wm[i].bitcast(f32r))
                    zps = ps_s.tile([P, 512], f32, tag="S", name=f"zps{qc}_{i}")
                    for j in range(8):
                        nc.tensor.matmul(
                            zps[:],
                            wmt[:, j, :].bitcast(f32r),
                            y_sb[:, j, :].bitcast(f32r),
                            start=(j == 0),
                            stop=(j == 7),
                        )
                    nc.any.tensor_copy(out=z_sb[:, i, :].bitcast(f32r), in_=zps[:])

                # ---- wo partial projection for this t-chunk ----
                for ft in range(16):
                    wot = w2_pool.tile([P, 8, P], f32, tag="wo", name=f"wo{qc}_{ft}")
                    eng = _eng(nc, ft)
                    eng.dma_start(wot[:].bitcast(f32r), wo[ft].bitcast(f32r))
                    ops = ps_t.tile([P, 512], f32, tag="tp")
                    for kt in range(8):
                        nc.tensor.matmul(
                            ops[:],
                            wot[:, kt, :].bitcast(f32r),
                            z_sb[:, kt, :].bitcast(f32r),
                            start=(kt == 0),
                            stop=(kt == 7),
                        )
                    osb = stage_pool.tile([P, 512], f32, tag="vstage", name=f"osb{qc}_{ft}")
                    nc.any.tensor_copy(out=osb[:], in_=ops[:])
                    eng = _eng(nc, ft)
                    eng.dma_start(outT[ft * P : (ft + 1) * P, tsl], osb[:])

            zp_cm.__exit__(None, None, None)
            w2_cm.__exit__(None, None, None)
            vh_cm.__exit__(None, None, None)
            pt_cm.__exit__(None, None, None)
            p_cm.__exit__(None, None, None)
            yp_cm.__exit__(None, None, None)
            qks_cm.__exit__(None, None, None)

    nc.finalize()
    _NC_CACHE[key] = nc
    return nc


def _run(inputs: dict, trace: bool = False):
    from concourse.bass_utils import run_bass_kernel_spmd

    wq_q = _ternary_quantize(np.asarray(inputs["wq"], dtype=np.float32))
    wk_q = _ternary_quantize(np.asarray(inputs["wk"], dtype=np.float32))
    wv_q = _ternary_quantize(np.asarray(inputs["wv"], dtype=np.float32))
    wo_q = _ternary_quantize(np.asarray(inputs["wo"], dtype=np.float32))

    in_maps = []
    for c in range(NCORES):
        b, g = c // 2, c % 2
        in_maps.append(_prep_core_inputs(inputs, b, g, wq_q, wk_q, wv_q, wo_q))

    nc = _build_nc()
    res = run_bass_kernel_spmd(nc, in_maps, list(range(NCORES)), trace=trace)

    out = np.empty((B, T, C), dtype=np.float32)
    for b in range(B):
        acc = res.results[2 * b]["outT"] + res.results[2 * b + 1]["outT"]
        out[b] = acc.T
    return out, res


def kernel(**inputs) -> np.ndarray:
    out, _ = _run(inputs, trace=False)
    return out



# revision 13
# speedup vs baseline: 1.4212x; 1.4212x over previous
"""Trainium2 Bass kernel for octonion causal self-attention (bf16 redesign).

Sharding: 8 cores = 4 batches x 2 head-groups. Core c handles batch b=c//2 and
head-group g=c%2 (octonion output components 4g..4g+3 = heads 8g..8g+7).
Each core computes q/k/v projections for its components from the full x[b],
RoPE, causal attention for its 8 heads, the octonion head-mixer for its group,
and a partial wo projection (its 4 input components, all 2048 output channels).
The host sums the two bf16 partials per batch and transposes. No collectives.

All matmuls run in bf16 (same PE rate as f32r on trn2, but half the DMA/SBUF
traffic and 2x DVE throughput). PSUM accumulation stays f32. V tiles stay in
SBUF (no DRAM spill). Causal structure is exploited: S matmuls, exp and PV
are trimmed to the lower triangle at 128-column granularity; the mask add
only touches the 128x128 diagonal block. Probs transposes are regular
128-free matmuls (stationary = exp'd scores) whose moving operand is
diag(1/l), folding the softmax normalization into the transpose for free.
"""

import math
import os
from contextlib import ExitStack

import numpy as np

B, T, C, H, D = 4, 1024, 2048, 16, 128
C8 = C // 8  # 256
NCORES = 8
P = 128
NEGM = -30000.0


# ---------------- octonion tables (matches reference) ----------------
def _cd_conj(a):
    n = a.shape[0]
    if n == 1:
        return a
    h = n // 2
    return np.concatenate([_cd_conj(a[:h]), -a[h:]])


def _cd_mul(a, b):
    n = a.shape[0]
    if n == 1:
        return a * b
    h = n // 2
    a1, a2 = a[:h], a[h:]
    c1, c2 = b[:h], b[h:]
    return np.concatenate(
        [
            _cd_mul(a1, c1) - _cd_mul(_cd_conj(c2), a2),
            _cd_mul(c2, a1) + _cd_mul(a2, _cd_conj(c1)),
        ]
    )


def _octonion_tables():
    signs = np.zeros((8, 8), dtype=np.float32)
    widx = np.zeros((8, 8), dtype=np.int32)
    for i in range(8):
        for j in range(8):
            ei = np.zeros(8)
            ei[i] = 1.0
            ej = np.zeros(8)
            ej[j] = 1.0
            p = _cd_mul(ei, ej)
            k = int(np.argmax(np.abs(p)))
            signs[i, j] = np.sign(p[k])
            widx[i, j] = k
    return signs, widx


SIGNS, WIDX = _octonion_tables()


def _bf16(a):
    import ml_dtypes

    return np.asarray(a, dtype=np.float32).astype(ml_dtypes.bfloat16)


def _ternary_quantize(W: np.ndarray) -> np.ndarray:
    """Replicates reference ternary_ste forward pass bit-exactly (jnp on CPU)."""
    import jax
    import jax.numpy as jnp

    with jax.default_device(jax.devices("cpu")[0]):
        Wj = jnp.asarray(W)
        s = jnp.mean(jnp.abs(Wj), axis=(-2, -1), keepdims=True) + 1e-8
        Wq = jnp.clip(jnp.round(Wj / s), -1.0, 1.0) * s
        return np.asarray(Wq)


def _signed_full(Wq: np.ndarray, i: int) -> np.ndarray:
    """[2048, 256] block column for octonion output component i:
    rows j*256:(j+1)*256 = SIGNS[i,j] * Wq[i^j]."""
    out = np.empty((C, C8), dtype=np.float32)
    for j in range(8):
        out[j * C8 : (j + 1) * C8, :] = SIGNS[i, j] * Wq[i ^ j]
    return out


def _prep_core_inputs(inputs: dict, b: int, g: int, wq_q, wk_q, wv_q, wo_q):
    x = inputs["x"]
    fc, fs = inputs["freqs_cos"], inputs["freqs_sin"]
    mixer_W, mixer_beta = inputs["mixer_W"], inputs["mixer_beta"]

    m = {}
    # x transposed, partition-major: [p, ct, t] bf16 (one big DMA)
    m["xT"] = _bf16(np.ascontiguousarray(x[b].T).reshape(16, P, T).transpose(1, 0, 2))

    # q/k weights: [qk, li, dh, c_p, ct, d], interleaved rope layout
    wqk = np.empty((2, 4, 2, P, 16, P), dtype=np.float32)
    qscale = np.float32(1.0 / math.sqrt(D))
    for qk, Wq in enumerate((wq_q, wk_q)):
        for li in range(4):
            i = 4 * g + li
            Bf = _signed_full(Wq, i)  # [2048, 256]
            if qk == 0:
                Bf = Bf * qscale
            for dh in range(2):
                Bh = Bf[:, dh * D : (dh + 1) * D]  # [2048, 128]
                wqk[qk, li, dh] = Bh.reshape(16, P, P).transpose(1, 0, 2)
    m["wqk"] = _bf16(wqk)

    # v weights: [lp, ct, c_p, dcol] with dcol = 2 comps x 256, natural order
    wv = np.empty((2, 16, P, 512), dtype=np.float32)
    for lp in range(2):
        B2 = np.concatenate(
            [_signed_full(wv_q, 4 * g + 2 * lp + u) for u in range(2)], axis=1
        )  # [2048, 512]
        wv[lp] = B2.reshape(16, P, 512)
    m["wv"] = _bf16(wv.transpose(2, 0, 1, 3))  # [p, lp, ct, d]

    # wo: [ft, d_p, kt, f] ; kt = local input channel tile (4 comps x 2 halves)
    wo = np.empty((16, P, 8, P), dtype=np.float32)
    for ft in range(16):
        i_o, fh = ft // 2, ft % 2
        for kt in range(8):
            j = 4 * g + kt // 2
            dloc = kt % 2
            blk = SIGNS[i_o, j] * wo_q[i_o ^ j]  # [256, 256]
            wo[ft, :, kt, :] = blk[dloc * P : (dloc + 1) * P, fh * P : (fh + 1) * P]
    m["wo"] = _bf16(wo.transpose(1, 0, 2, 3))  # [p, ft, kt, f]

    # mixer: [i, d_p, j, e], beta folded on output dim
    wm = np.empty((8, P, 8, P), dtype=np.float32)
    for i in range(8):
        for j in range(8):
            wm[i, :, j, :] = (SIGNS[i, j] * mixer_W[i ^ j]) * mixer_beta[None, :]
    m["wm"] = _bf16(wm.transpose(1, 0, 2, 3))  # [p, i, j, e]

    # interleaved rope layout: row p uses freq p//2; partner = p xor 1 via PE
    cosA = np.repeat(np.ascontiguousarray(fc.T), 2, axis=0)
    sinB = np.repeat(np.ascontiguousarray(fs.T), 2, axis=0)
    sinB[0::2, :] *= -1.0
    m["cosd"] = _bf16(cosA)
    m["sind"] = _bf16(sinB)
    pswap = np.zeros((P, P), dtype=np.float32)
    for p in range(P):
        pswap[p ^ 1, p] = 1.0
    m["pswap"] = _bf16(pswap)
    m["ident"] = _bf16(np.eye(P, dtype=np.float32))

    # causal mask for the 128x128 diagonal block: allow col j <= row p
    pidx = np.arange(P)[:, None]
    jidx = np.arange(P)[None, :]
    m["tri"] = _bf16(np.where(jidx <= pidx, 0.0, NEGM).astype(np.float32))
    return m


# ---------------- device program ----------------
_NC_CACHE = {}


def _build_nc(repeat: int = 1, pp_bufs: int = 4, attn_depth: int = 2):
    key = (repeat, pp_bufs, attn_depth)
    if key in _NC_CACHE:
        return _NC_CACHE[key]

    import concourse.mybir as mybir
    import concourse.tile as tile
    from concourse import bacc

    dt = mybir.dt
    ALU = mybir.AluOpType
    AF = mybir.ActivationFunctionType
    f32, bf16 = dt.float32, dt.bfloat16

    nc = bacc.Bacc("TRN2", target_bir_lowering=False)

    xT = nc.declare_dram_parameter("xT", [P, 16, T], bf16, isOutput=False)
    wqk = nc.declare_dram_parameter("wqk", [2, 4, 2, P, 16, P], bf16, isOutput=False)
    wv = nc.declare_dram_parameter("wv", [P, 2, 16, 512], bf16, isOutput=False)
    wo = nc.declare_dram_parameter("wo", [P, 16, 8, P], bf16, isOutput=False)
    wm = nc.declare_dram_parameter("wm", [P, 8, 8, P], bf16, isOutput=False)
    cosd = nc.declare_dram_parameter("cosd", [P, T], bf16, isOutput=False)
    sind = nc.declare_dram_parameter("sind", [P, T], bf16, isOutput=False)
    trip = nc.declare_dram_parameter("tri", [P, P], bf16, isOutput=False)
    identp = nc.declare_dram_parameter("ident", [P, P], bf16, isOutput=False)
    pswapp = nc.declare_dram_parameter("pswap", [P, P], bf16, isOutput=False)
    outT = nc.declare_dram_parameter("outT", [C, T], bf16, isOutput=True)

    with tile.TileContext(nc) as tc, ExitStack() as ctx:
        cst = ctx.enter_context(tc.tile_pool(name="cst", bufs=1))
        statp = ctx.enter_context(tc.tile_pool(name="statp", bufs=16))
        xp = ctx.enter_context(tc.tile_pool(name="xp", bufs=1))
        wqk_pool = ctx.enter_context(tc.tile_pool(name="wqkp", bufs=2))
        ropeA = ctx.enter_context(tc.tile_pool(name="ropeA", bufs=4))
        ropeB = ctx.enter_context(tc.tile_pool(name="ropeB", bufs=1))
        qks = ctx.enter_context(tc.tile_pool(name="qks", bufs=1))
        vsb = ctx.enter_context(tc.tile_pool(name="vsb", bufs=1))
        # PSUM pools: pst+psy always open (4 banks); pp (proj, 4 banks) and
        # pss (S, [128,1024] = 2 banks x 2 bufs) are phase-local.
        ps_t = ctx.enter_context(tc.tile_pool(name="pst", bufs=2, space="PSUM"))
        ps_y = ctx.enter_context(tc.tile_pool(name="psy", bufs=2, space="PSUM"))

        # constants loaded once (gpsimd SWDGE queue)
        ident = cst.tile([P, P], bf16, tag="ident")
        nc.gpsimd.dma_start(ident[:], identp[:])
        pswap_sb = cst.tile([P, P], bf16, tag="pswap")
        nc.gpsimd.dma_start(pswap_sb[:], pswapp[:])
        cos_sb = cst.tile([P, T], bf16, tag="cos")
        nc.gpsimd.dma_start(cos_sb[:], cosd[:])
        sin_sb = cst.tile([P, T], bf16, tag="sin")
        nc.gpsimd.dma_start(sin_sb[:], sind[:])
        tri_sb = cst.tile([P, P], bf16, tag="tri")
        nc.gpsimd.dma_start(tri_sb[:], trip[:])

        for _rep in range(repeat):
            qT_h = [qks.tile([P, T], bf16, tag=f"qT{i}", name=f"qTh{i}") for i in range(8)]
            kT_h = [qks.tile([P, T], bf16, tag=f"kT{i}", name=f"kTh{i}") for i in range(8)]
            v_t = [
                [vsb.tile([P, 512], bf16, tag=f"v{lp}_{tt}", name=f"vt{lp}_{tt}") for tt in range(8)]
                for lp in range(2)
            ]
            wv_cm = tc.tile_pool(name="wvp", bufs=1)
            wv_pool = wv_cm.__enter__()

            pp_cm = tc.tile_pool(name="pp", bufs=pp_bufs, space="PSUM")
            pp = pp_cm.__enter__()

            # batched x + wv loads (2 big DMAs each, on separate queues)
            x_all = xp.tile([P, 16, T], bf16, tag="xall", name="xall")
            nc.sync.dma_start(x_all[:, 0:8, :], xT[:, 0:8, :])
            nc.sync.dma_start(x_all[:, 8:16, :], xT[:, 8:16, :])
            x_t = [x_all[:, ct, :] for ct in range(16)]
            wv_all = wv_pool.tile([P, 2, 16, 512], bf16, tag="wvall", name="wvall")
            nc.gpsimd.dma_start(wv_all[:, 0], wv[:, 0])
            nc.gpsimd.dma_start(wv_all[:, 1], wv[:, 1])
            wv_tiles = {(lp, ct): wv_all[:, lp, ct, :] for lp in range(2) for ct in range(16)}

            # ---- Q/K projections with fused RoPE (pswap pipelined 1 iter) ----
            def emit_rope(dest, hh, qsbs):
                for tci in range(2):
                    tsl = slice(tci * 512, (tci + 1) * 512)
                    qsb = qsbs[tci]
                    swps = ps_t.tile([P, 512], f32, tag="tp")
                    nc.tensor.matmul(
                        swps[:], pswap_sb[:], qsb[:], start=True, stop=True
                    )
                    qsw = ropeB.tile([P, 512], bf16, tag="qsw")
                    nc.vector.tensor_copy(out=qsw[:], in_=swps[:])
                    t1 = ropeB.tile([P, 512], bf16, tag="t1")
                    t2 = ropeB.tile([P, 512], bf16, tag="t2")
                    nc.vector.tensor_tensor(t1[:], qsb[:], cos_sb[:, tsl], ALU.mult)
                    nc.vector.tensor_tensor(t2[:], qsw[:], sin_sb[:, tsl], ALU.mult)
                    nc.vector.tensor_tensor(dest[hh][:, tsl], t1[:], t2[:], ALU.add)

            rope_pending = None
            for qk, dest_h in ((0, qT_h), (1, kT_h)):
                for li in range(4):
                    for dh in range(2):
                        hh = li * 2 + dh
                        wt = wqk_pool.tile([P, 16, P], bf16, tag="wqk")
                        eng = nc.sync if (hh % 2 == 0) else nc.gpsimd
                        eng.dma_start(wt[:], wqk[qk, li, dh])
                        pps = [
                            pp.tile([P, 512], f32, tag="proj", name=f"pp{t}")
                            for t in range(2)
                        ]
                        for ct in range(16):
                            for tci in range(2):
                                nc.tensor.matmul(
                                    pps[tci][:],
                                    wt[:, ct, :],
                                    x_all[:, ct, tci * 512 : (tci + 1) * 512],
                                    start=(ct == 0),
                                    stop=(ct == 15),
                                )
                        qsbs = []
                        for tci in range(2):
                            qsb = ropeA.tile([P, 512], bf16, tag="qsb")
                            nc.vector.tensor_copy(out=qsb[:], in_=pps[tci][:])
                            qsbs.append(qsb)
                        if rope_pending is not None:
                            emit_rope(*rope_pending)
                        rope_pending = (dest_h, hh, qsbs)
            emit_rope(*rope_pending)

            # ---- V projection (stays in SBUF) ----
            for lp in range(2):
                for tt in range(8):
                    vps = pp.tile([P, 512], f32, tag="proj", name=f"vp{lp}_{tt}")
                    for ct in range(16):
                        nc.tensor.matmul(
                            vps[:],
                            x_all[:, ct, tt * P : (tt + 1) * P],
                            wv_all[:, lp, ct, :],
                            start=(ct == 0),
                            stop=(ct == 15),
                        )
                    nc.scalar.copy(out=v_t[lp][tt][:], in_=vps[:])

            wv_cm.__exit__(None, None, None)
            pp_cm.__exit__(None, None, None)
            pss_cm = tc.tile_pool(name="pss", bufs=2, space="PSUM")
            ps_s = pss_cm.__enter__()

            # attention-phase pools
            psb_cm = tc.tile_pool(name="psb", bufs=12)
            psb = psb_cm.__enter__()
            diag_cm = tc.tile_pool(name="diagp", bufs=12)
            diagp = diag_cm.__enter__()
            pt_cm = tc.tile_pool(name="ptsb", bufs=9)
            ptsb = pt_cm.__enter__()
            y_cm = tc.tile_pool(name="ysb", bufs=1)
            yp = y_cm.__enter__()
            z_cm = tc.tile_pool(name="zsb", bufs=1)
            zp = z_cm.__enter__()
            wm_cm = tc.tile_pool(name="wmp", bufs=1)
            wmp = wm_cm.__enter__()
            wo_cm = tc.tile_pool(name="wop", bufs=1)
            wop = wo_cm.__enter__()
            out_cm = tc.tile_pool(name="outp", bufs=1)
            outp = out_cm.__enter__()

            wm_all = wmp.tile([P, 8, 8, P], bf16, tag="wmall", name="wmall")
            nc.sync.dma_start(wm_all[:], wm[:])
            wm_t = [wm_all[:, i] for i in range(8)]
            wo_all = wop.tile([P, 16, 8, P], bf16, tag="woall", name="woall")
            nc.gpsimd.dma_start(wo_all[:, 0:8], wo[:, 0:8])
            nc.gpsimd.dma_start(wo_all[:, 8:16], wo[:, 8:16])
            wo_t = [wo_all[:, ft] for ft in range(16)]

            def emit_wo_ft(ft, z_src, tsl_prev, osb_box):
                if ft % 4 == 0:
                    osb_box[0] = outp.tile(
                        [P, 4, 512], bf16, tag="osb", name=f"osbd{ft}_{_rep}"
                    )
                osb = osb_box[0]
                ops = ps_t.tile([P, 512], f32, tag="tp")
                for kt in range(8):
                    nc.tensor.matmul(
                        ops[:],
                        wo_t[ft][:, kt, :],
                        z_src[:, kt, :],
                        start=(kt == 0),
                        stop=(kt == 7),
                    )
                nc.scalar.copy(out=osb[:, ft % 4, :], in_=ops[:])
                if ft % 4 == 3:
                    f0 = ft - 3
                    nc.scalar.dma_start(
                        outT[f0 * P : (f0 + 4) * P, tsl_prev].rearrange(
                            "(f p) t -> p f t", p=P
                        ),
                        osb[:],
                    )

            def emit_S(h, qc):
                """S matmuls + diag mask + exp + recip + diag(r) for 4 q-blocks."""
                Ps_list = {}
                diag_list = {}
                for qt in range(4 * qc, 4 * qc + 4):
                    wq_w = (qt + 1) * P
                    Ps = psb.tile([P, T], bf16, tag="P", name=f"Ps{qc}_{h}_{qt}")
                    Ps_list[qt] = Ps
                    sps = ps_s.tile([P, T], f32, tag="S")
                    for chi in range((wq_w + 511) // 512):
                        w = min(512, wq_w - chi * 512)
                        nc.tensor.matmul(
                            sps[:, chi * 512 : chi * 512 + w],
                            qT_h[h][:, qt * P : (qt + 1) * P],
                            kT_h[h][:, chi * 512 : chi * 512 + w],
                            start=True,
                            stop=False,
                            skip_group_check=True,
                        )
                    nc.tensor.matmul(
                        sps[:, wq_w - P : wq_w],
                        ident[:],
                        tri_sb[:],
                        start=False,
                        stop=True,
                        skip_group_check=True,
                    )
                    ltot = statp.tile([P, 1], f32, tag="l")
                    nc.scalar.activation(
                        Ps[:, :wq_w], sps[:, :wq_w], AF.Exp, accum_out=ltot[:]
                    )
                    rec = statp.tile([P, 1], f32, tag="r")
                    nc.vector.reciprocal(rec[:], ltot[:])
                    dg = diagp.tile([P, P], bf16, tag="diag", name=f"dg{qc}_{h}_{qt}")
                    nc.vector.tensor_scalar(
                        dg[:], ident[:], rec[:], None, op0=ALU.mult
                    )
                    diag_list[qt] = dg
                return Ps_list, diag_list

            def emit_TPV(h, qc, Ps_list, diag_list, y_sb):
                """Normalizing transposes (regular matmuls vs diag(1/l)) + PV."""
                nkt = 4 * (qc + 1)
                pts = []
                for kt in range(nkt):
                    qt0 = max(kt, 4 * qc)
                    off = (qt0 - 4 * qc) * P
                    ptps = ps_t.tile([P, 512], f32, tag="tp")
                    for qt in range(qt0, 4 * qc + 4):
                        cl = (qt % 4) * P
                        nc.tensor.matmul(
                            ptps[:, cl : cl + P],
                            Ps_list[qt][:, kt * P : (kt + 1) * P],
                            diag_list[qt][:],
                            start=True,
                            stop=True,
                        )
                    pt_sb = ptsb.tile([P, 512], bf16, tag="PT", name=f"PT{qc}_{h}_{kt}")
                    nc.vector.tensor_copy(out=pt_sb[:, off:], in_=ptps[:, off:])
                    pts.append((pt_sb, off))
                yps = ps_y.tile([P, 512], f32, tag="y")
                lp, dcol = h // 4, (h % 4) * P
                for kt in range(nkt):
                    pt_sb, off = pts[kt]
                    nc.tensor.matmul(
                        yps[:, off:],
                        v_t[lp][kt][:, dcol : dcol + P],
                        pt_sb[:, off:],
                        start=(kt == 0),
                        stop=(kt == nkt - 1),
                        skip_group_check=True,
                    )
                nc.scalar.copy(out=y_sb[:, h, :], in_=yps[:])

            deferred_wo = None
            for qc in range(2):
                tsl = slice(qc * 512, (qc + 1) * 512)
                y_sb = yp.tile([P, 8, 512], bf16, tag="y", name=f"ysb{qc}")
                z_sb = zp.tile([P, 8, 512], bf16, tag="z", name=f"zsb{qc}")
                osb_box = [None]
                pending = []
                for h in range(8):
                    pending.append((h, emit_S(h, qc)))
                    if deferred_wo is not None:
                        z_prev, tsl_prev = deferred_wo
                        emit_wo_ft(2 * h, z_prev, tsl_prev, osb_box)
                        emit_wo_ft(2 * h + 1, z_prev, tsl_prev, osb_box)
                    if len(pending) > attn_depth:
                        ph, cur = pending.pop(0)
                        emit_TPV(ph, qc, cur[0], cur[1], y_sb)
                deferred_wo = None
                for ph, cur in pending:
                    emit_TPV(ph, qc, cur[0], cur[1], y_sb)

                # ---- head mixer for this t-chunk ----
                for i in range(8):
                    zps = ps_y.tile([P, 512], f32, tag="y", name=f"zps{qc}_{i}")
                    for j in range(8):
                        nc.tensor.matmul(
                            zps[:],
                            wm_t[i][:, j, :],
                            y_sb[:, j, :],
                            start=(j == 0),
                            stop=(j == 7),
                        )
                    nc.scalar.copy(out=z_sb[:, i, :], in_=zps[:])

                # ---- wo partial projection: qc0 deferred into qc1's
                # attention pipeline; qc1 emitted inline ----
                if qc == 0:
                    deferred_wo = (z_sb, tsl)
                else:
                    box = [None]
                    for ft in range(16):
                        emit_wo_ft(ft, z_sb, tsl, box)

            pss_cm.__exit__(None, None, None)
            out_cm.__exit__(None, None, None)
            wo_cm.__exit__(None, None, None)
            wm_cm.__exit__(None, None, None)
            z_cm.__exit__(None, None, None)
            y_cm.__exit__(None, None, None)
            pt_cm.__exit__(None, None, None)
            diag_cm.__exit__(None, None, None)
            psb_cm.__exit__(None, None, None)

    nc.finalize()
    _NC_CACHE[key] = nc
    return nc


def _run(inputs: dict, trace: bool = False):
    from concourse.bass_utils import run_bass_kernel_spmd

    wq_q = _ternary_quantize(np.asarray(inputs["wq"], dtype=np.float32))
    wk_q = _ternary_quantize(np.asarray(inputs["wk"], dtype=np.float32))
    wv_q = _ternary_quantize(np.asarray(inputs["wv"], dtype=np.float32))
    wo_q = _ternary_quantize(np.asarray(inputs["wo"], dtype=np.float32))

    in_maps = []
    for c in range(NCORES):
        b, g = c // 2, c % 2
        in_maps.append(_prep_core_inputs(inputs, b, g, wq_q, wk_q, wv_q, wo_q))

    nc = _build_nc()
    res = run_bass_kernel_spmd(nc, in_maps, list(range(NCORES)), trace=trace)

    out = np.empty((B, T, C), dtype=np.float32)
    for b in range(B):
        acc = np.asarray(res.results[2 * b]["outT"]).astype(np.float32) + np.asarray(
            res.results[2 * b + 1]["outT"]
        ).astype(np.float32)
        out[b] = acc.T
    return out, res


def kernel(**inputs) -> np.ndarray:
    out, _ = _run(inputs, trace=False)
    return out
